# revision 5
# baseline (speedup 1.0000x reference)
"""VQ cosine-sim codebook kernel (nn_CosineSimCodebook) for 8 TRN2 NeuronCores.

Contract: kernel(x=[16,2048,512] f32, embed=[1,8192,512] f32) ->
          (quantize [16,2048,512] f32, embed_ind [16,2048] i32)

Data-parallel: 32768 tokens sharded 4096/core across 8 cores; codebook
replicated. Per-core two-phase argmax:
  Phase A: fp16 x_hi @ e_hi^T on TensorE -> approx dist [128, 8192] per
           128-token tile (PSUM f32, cast to fp16 in SBUF via ScalarE).
  Top-8:   VectorE InstMax + InstMaxIndex per token row.
  Phase B: SWDGE dma_gather of the 8 candidate codebook rows per token,
           exact fp32 re-score on VectorE (tensor_tensor_reduce), argmax
           with first-index tie-break, final gather of the winning row.
"""

import os
import sys
from contextlib import ExitStack

sys.path.insert(0, "/opt/trn_rl_repo")

import numpy as np

import concourse.bacc as bacc
import concourse.bass as bass
import concourse.mybir as mybir
from concourse import tile
from concourse.bass_utils import run_bass_kernel_spmd

dt = mybir.dt
Alu = mybir.AluOpType

P = 128
D = 512
KC = 4          # contraction chunks (D / 128)
C = 8192        # codebook size
NCT = C // 512  # code tiles of 512
N_CORES = 8
T = 4096        # tokens per core (32768 / 8)


def _declare_io(nc):
    xt16 = nc.dram_tensor("xt16", [KC, P, T], dt.float16, kind="ExternalInput").ap()
    x32 = nc.dram_tensor("x32", [T, D], dt.float32, kind="ExternalInput").ap()
    e16t = nc.dram_tensor("e16t", [KC, P, C], dt.float16, kind="ExternalInput").ap()
    e32 = nc.dram_tensor("e32", [C, D], dt.float32, kind="ExternalInput").ap()
    quant = nc.dram_tensor("quant", [T, D], dt.float32, kind="ExternalOutput").ap()
    ind = nc.dram_tensor("ind", [T], dt.int32, kind="ExternalOutput").ap()
    return xt16, x32, e16t, e32, quant, ind


def _build_kernel(tc, io):
    nc = tc.nc
    xt16, x32, e16t, e32, quant, ind = io
    NT = T // P

    with ExitStack() as ctx:
        const = ctx.enter_context(tc.tile_pool(name="const", bufs=1))
        xpool = ctx.enter_context(tc.tile_pool(name="x", bufs=3))
        dpool = ctx.enter_context(tc.tile_pool(name="dist", bufs=2))
        psum = ctx.enter_context(tc.tile_pool(name="ps", bufs=8, space="PSUM"))
        small = ctx.enter_context(tc.tile_pool(name="small", bufs=3))
        cpool = ctx.enter_context(tc.tile_pool(name="cand", bufs=2))
        qpool = ctx.enter_context(tc.tile_pool(name="q", bufs=3))
        dram = ctx.enter_context(tc.tile_pool(name="dram", bufs=4, space="DRAM"))

        e_sb = const.tile([P, KC, C], dt.float16)
        nc.sync.dma_start(e_sb[:], e16t.rearrange("k p c -> p k c"))

        for ti in range(NT):
            tok = bass.ts(ti, P)

            xt = xpool.tile([P, KC, P], dt.float16, tag="xt")
            nc.sync.dma_start(xt[:], xt16[:, :, tok].rearrange("k p m -> p k m"))
            xr = xpool.tile([P, D], dt.float32, tag="xr")
            nc.sync.dma_start(xr[:], x32[tok, :])

            dist = dpool.tile([P, NCT, 512], dt.float16, tag="dist")
            for cj in range(NCT):
                ps = psum.tile([P, 512], dt.float32, tag="ps")
                for kc in range(KC):
                    nc.tensor.matmul(
                        ps[:],
                        xt[:, kc, :],
                        e_sb[:, kc, bass.ts(cj, 512)],
                        start=(kc == 0),
                        stop=(kc == KC - 1),
                    )
                nc.scalar.copy(dist[:, cj, :], ps[:])

            dist2d = dist[:].rearrange("p a b -> p (a b)")

            vmax = small.tile([P, 8], dt.float16, tag="vmax")
            nc.vector.max(vmax[:], dist2d)
            idx8 = small.tile([P, 8], dt.uint16, tag="idx8")
            nc.vector.max_index(idx8[:], vmax[:], dist2d)

            # wrapped idx layout for dma_gather: idxs[p, 8j+b] = idx8[16b+p, j]
            idxd = dram.tile([P, 8], dt.uint16, tag="idxd")
            nc.sync.dma_start(idxd[:], idx8[:])
            idxs = small.tile([P, 64], dt.int16, tag="idxs")
            src_w = idxd[:].bitcast(dt.int16).rearrange("(b p) j -> p j b", p=16)
            for r in range(8):
                nc.sync.dma_start(
                    idxs[bass.ts(r, 16), :].rearrange("p (j b) -> p j b", j=8),
                    src_w,
                )

            cand = cpool.tile([P, 8, D], dt.float32, tag="cand")
            nc.gpsimd.dma_gather(cand[:], e32, idxs[:], 8 * P, 8 * P, D)

            resc = small.tile([P, 8], dt.float32, tag="resc")
            prod8 = cpool.tile([P, 8, D], dt.float32, tag="prod8")
            nc.vector.tensor_tensor(
                prod8[:],
                cand[:],
                xr[:].rearrange("p (one d) -> p one d", one=1).broadcast_to([P, 8, D]),
                Alu.mult,
            )
            nc.vector.tensor_reduce(
                resc[:], prod8[:], axis=mybir.AxisListType.X, op=Alu.add
            )

            rmax = small.tile([P, 1], dt.float32, tag="rmax")
            nc.vector.tensor_reduce(rmax[:], resc[:], axis=mybir.AxisListType.X, op=Alu.max)
            ismax = small.tile([P, 8], dt.float32, tag="ismax")
            nc.vector.tensor_scalar(ismax[:], resc[:], rmax[:], None, Alu.is_equal)
            idxf = small.tile([P, 8], dt.float32, tag="idxf")
            nc.vector.tensor_copy(idxf[:], idx8[:])
            key = small.tile([P, 8], dt.float32, tag="key")
            nc.vector.tensor_scalar(key[:], idxf[:], -1.0, float(C), Alu.mult, Alu.add)
            key2 = small.tile([P, 8], dt.float32, tag="key2")
            nc.vector.tensor_tensor(key2[:], key[:], ismax[:], Alu.mult)
            nkey = small.tile([P, 1], dt.float32, tag="nkey")
            nc.vector.tensor_reduce(nkey[:], key2[:], axis=mybir.AxisListType.X, op=Alu.max)
            fidx = small.tile([P, 1], dt.float32, tag="fidx")
            nc.vector.tensor_scalar(fidx[:], nkey[:], -1.0, float(C), Alu.mult, Alu.add)

            iout = small.tile([P, 1], dt.int32, tag="iout")
            nc.vector.tensor_copy(iout[:], fidx[:])
            nc.sync.dma_start(ind[tok].rearrange("(p one) -> p one", one=1), iout[:])

            fu16 = small.tile([P, 1], dt.uint16, tag="fu16")
            nc.vector.tensor_copy(fu16[:], fidx[:])
            idx2d = dram.tile([P, 1], dt.uint16, tag="idx2d")
            nc.sync.dma_start(idx2d[:], fu16[:])
            idxs2 = small.tile([P, 8], dt.int16, tag="idxs2")
            src2 = idx2d[:].bitcast(dt.int16).rearrange("(s p) one -> p (s one)", p=16)
            for r in range(8):
                nc.sync.dma_start(idxs2[bass.ts(r, 16), :], src2)
            qt = qpool.tile([P, 1, D], dt.float32, tag="qt")
            nc.gpsimd.dma_gather(qt[:], e32, idxs2[:], P, P, D)
            nc.sync.dma_start(quant[tok, :], qt[:, 0, :])


def _timed_run(in_maps, iters=8, nc=None):
    """Steady-state timing: jit one bass_exec (shard_map over the cores),
    device-put inputs once, pre-stage donated zero out-buffers, time repeated
    executions. Returns (min_wall_ns, results)."""
    import time

    import jax
    from jax.sharding import Mesh, PartitionSpec
    from jax.experimental.shard_map import shard_map

    from concourse import bass2jax
    from concourse.bass2jax import _bass_exec_p, partition_id_tensor

    bass2jax.install_neuronx_cc_hook()
    if nc is None:
        nc = _get_nc()

    partition_name = nc.partition_id_tensor.name if nc.partition_id_tensor else None
    in_names, out_names, out_avals, zero_outs = [], [], [], []
    for alloc in nc.m.functions[0].allocations:
        if not isinstance(alloc, mybir.MemoryLocationSet):
            continue
        name = alloc.memorylocations[0].name
        if alloc.kind == "ExternalInput":
            if name != partition_name:
                in_names.append(name)
        elif alloc.kind == "ExternalOutput":
            out_names.append(name)
            shape = tuple(alloc.tensor_shape)
            dtype = mybir.dt.np(alloc.dtype)
            out_avals.append(jax.core.ShapedArray(shape, dtype))
            zero_outs.append(np.zeros(shape, dtype))
    n_params = len(in_names)
    n_outs = len(out_avals)
    all_in_names = list(in_names) + list(out_names)
    if partition_name is not None:
        all_in_names.append(partition_name)

    def _one(args):
        operands = list(args)
        if partition_name is not None:
            operands.append(partition_id_tensor())
        return _bass_exec_p.bind(
            *operands,
            out_avals=tuple(out_avals),
            in_names=tuple(all_in_names),
            out_names=tuple(out_names),
            lowering_input_output_aliases=(),
            sim_require_finite=True,
            sim_require_nnan=True,
            nc=nc,
        )

    def _body(*args):
        return tuple(_one(list(args)))

    devices = jax.devices()[: len(in_maps)]
    mesh = Mesh(np.asarray(devices), ("core",))
    in_specs = (PartitionSpec("core"),) * (n_params + n_outs)
    out_specs = (PartitionSpec("core"),) * n_outs
    donate = tuple(range(n_params, n_params + n_outs))

    f1 = jax.jit(
        shard_map(
            _body, mesh=mesh, in_specs=in_specs, out_specs=out_specs, check_rep=False
        ),
        donate_argnums=donate,
        keep_unused=True,
    )

    concat_in = [
        np.concatenate([np.asarray(m[name]) for m in in_maps], axis=0)
        for name in in_names
    ]
    dev_in = [jax.device_put(a) for a in concat_in]

    def _zeros():
        return [
            jax.device_put(np.zeros((len(in_maps) * z.shape[0], *z.shape[1:]), z.dtype))
            for z in zero_outs
        ]

    # warmup (compile)
    out1 = f1(*dev_in, *_zeros())
    jax.block_until_ready(out1)

    ts = []
    for _ in range(iters):
        zs = _zeros()
        jax.block_until_ready(zs)
        t0 = time.perf_counter()
        out1 = f1(*dev_in, *zs)
        jax.block_until_ready(out1)
        ts.append(time.perf_counter() - t0)

    results = [
        {
            name: np.asarray(out1[i]).reshape(len(in_maps), *out_avals[i].shape)[c]
            for i, name in enumerate(out_names)
        }
        for c in range(len(in_maps))
    ]
    return min(ts) * 1e9, sorted(ts)[len(ts) // 2] * 1e9, results


_NC_CACHE = None


def _get_nc():
    global _NC_CACHE
    if _NC_CACHE is None:
        nc = bacc.Bacc("TRN2", target_bir_lowering=False, debug=False)
        io = _declare_io(nc)
        with tile.TileContext(nc) as tc:
            _build_kernel(tc, io)
        nc.compile()
        _NC_CACHE = nc
    return _NC_CACHE


def kernel(x, embed, _trace=False, _trace_kwargs=None):
    x = np.asarray(x, dtype=np.float32)
    e = np.ascontiguousarray(np.asarray(embed, dtype=np.float32)[0])  # [8192, 512]

    X = x.reshape(-1, D)  # [32768, 512]
    e16t = np.ascontiguousarray(e.T.astype(np.float16)).reshape(KC, P, C)

    in_maps = []
    for s in range(N_CORES):
        xs = np.ascontiguousarray(X[s * T : (s + 1) * T])
        xt16 = np.ascontiguousarray(xs.T.astype(np.float16)).reshape(KC, P, T)
        in_maps.append({"xt16": xt16, "x32": xs, "e16t": e16t, "e32": e})

    nc = _get_nc()
    kw = {}
    if _trace:
        kw = dict(trace=True, trace_kwargs=(_trace_kwargs or {}))
    res = run_bass_kernel_spmd(nc, in_maps, list(range(N_CORES)), **kw)

    quant = np.concatenate([r["quant"] for r in res.results], axis=0)
    ind = np.concatenate([r["ind"] for r in res.results], axis=0)
    out_q = quant.reshape(16, 2048, D).astype(np.float32)
    out_i = ind.reshape(16, 2048).astype(np.int32)
    kernel.last_results = res
    return out_q, out_i
